# revision 4
# baseline (speedup 1.0000x reference)
"""Trainium2 Bass kernel for nn_Attention_55499567399068 (v2.1).

Episode-attention, data-parallel over batch across 8 NeuronCores
(32 episodes => 256 (b, n) pairs per core), 32 superblocks of 8 pairs.

Structure (v2.1):
  - q/k projections from xbar-transposed bf16 inputs (one transpose DMA per
    tensor per superblock); v is consumed in natural layout (no v transpose,
    no v projection):
      B^T  = Xv^T A^T                      hidT = (Wr1 Wv) B^T + (br1+Wr1 bv)
      w    = lrelu(hidT)^T wr2 + br2       g = A^T w
      z    = Xv^T G (block-diag G)         out = Wv z + bv * sum(w)
  - Block-diagonal E: pair pr's scores/E live at partitions (pr%2)*64..,
    cols pr*64.. of a zeroed [128, 512] tile, so the transposed A^T is
    2-pair block-diagonal and B^T needs only 16 full-K matmuls; g needs 4.
  - Scalar engine runs only {Exp, Lrelu, Copy} (one act table set); the
    projection bias-adds run on DVE as tensor_scalar.
  - Output DMA issues from the DVE queue so the sync queue (transposes)
    never blocks on a superblock's tail.
"""

import sys

sys.path.insert(0, "/opt/trn_rl_repo")

import ml_dtypes
import numpy as np

import concourse.bass as bass
import concourse.tile as tile
from concourse import bacc, mybir
from concourse.bass_utils import run_bass_kernel_spmd

F32 = mybir.dt.float32
BF16 = mybir.dt.bfloat16
BF16_NP = ml_dtypes.bfloat16

BS, NWAY, NSHOT, D = 256, 8, 64, 512
NCORES = 8
BS_SH = BS // NCORES
NPAIR = BS_SH * NWAY
SUPER = 8
NSB = NPAIR // SUPER
ROWS_SB = SUPER * NSHOT
LEAK = 0.01
AT = mybir.ActivationFunctionType
ALU = mybir.AluOpType

BR2_VAL = [0.0]


def build_nc(repeat=1, n_sb=NSB):
    nc = bacc.Bacc("TRN2", target_bir_lowering=False)

    xq = nc.dram_tensor("xq", [NPAIR * NSHOT, D], F32, kind="ExternalInput")
    xk = nc.dram_tensor("xk", [NPAIR * NSHOT, D], F32, kind="ExternalInput")
    xv = nc.dram_tensor("xv", [NPAIR * NSHOT, D], F32, kind="ExternalInput")
    wqT_d = nc.dram_tensor("wqT", [D, D], BF16, kind="ExternalInput")
    wkT_d = nc.dram_tensor("wkT", [D, D], BF16, kind="ExternalInput")
    wvT_d = nc.dram_tensor("wvT", [D, D], BF16, kind="ExternalInput")
    w1vT_d = nc.dram_tensor("w1vT", [D, 64], BF16, kind="ExternalInput")
    wr2T_d = nc.dram_tensor("wr2T", [64, 1], BF16, kind="ExternalInput")
    bq_d = nc.dram_tensor("bq", [128, 4], F32, kind="ExternalInput")
    bk_d = nc.dram_tensor("bk", [128, 4], F32, kind="ExternalInput")
    b1c_d = nc.dram_tensor("b1c", [64, 1], F32, kind="ExternalInput")
    bvr_d = nc.dram_tensor("bvr", [1, D], BF16, kind="ExternalInput")
    ones_d = nc.dram_tensor("ones", [64, 1], BF16, kind="ExternalInput")
    out_d = nc.dram_tensor("out", [NPAIR, D], F32, kind="ExternalOutput")

    with tile.TileContext(nc) as tc:
        import contextlib

        ctx = contextlib.ExitStack()
        with ctx:
            const_pool = ctx.enter_context(tc.tile_pool(name="const", bufs=1))
            ld_pool = ctx.enter_context(tc.tile_pool(name="loads", bufs=3))
            v_pool = ctx.enter_context(tc.tile_pool(name="vload", bufs=2))
            xt_pool = ctx.enter_context(tc.tile_pool(name="xt", bufs=2))
            proj_pool = ctx.enter_context(tc.tile_pool(name="projs", bufs=2))
            mid_pool = ctx.enter_context(tc.tile_pool(name="mid", bufs=2))
            out_pool = ctx.enter_context(tc.tile_pool(name="outs", bufs=2))
            psA = ctx.enter_context(tc.tile_pool(name="psA", bufs=2, space="PSUM"))
            psS = ctx.enter_context(tc.tile_pool(name="psS", bufs=3, space="PSUM"))
            psC = ctx.enter_context(tc.tile_pool(name="psC", bufs=2, space="PSUM"))
            psB = ctx.enter_context(tc.tile_pool(name="psB", bufs=1, space="PSUM"))

            wqT = const_pool.tile([128, 4 * D], BF16, tag="wqT")
            wkT = const_pool.tile([128, 4 * D], BF16, tag="wkT")
            wvT = const_pool.tile([128, 4 * D], BF16, tag="wvT")
            w1vT = const_pool.tile([128, 4 * 64], BF16, tag="w1vT")
            wr2T = const_pool.tile([64, 1], BF16, tag="wr2T")
            bqs = const_pool.tile([128, 4], F32, tag="bqs")
            bks = const_pool.tile([128, 4], F32, tag="bks")
            b1c = const_pool.tile([64, 1], F32, tag="b1c")
            bvr = const_pool.tile([1, D], BF16, tag="bvr")
            ones = const_pool.tile([64, 1], BF16, tag="ones")

            def load_consts():
                nc.sync.dma_start(
                    wqT[:].rearrange("p (dc h) -> p dc h", dc=4),
                    wqT_d[:, :].rearrange("(dc p) h -> p dc h", p=128),
                )
                nc.sync.dma_start(
                    wkT[:].rearrange("p (dc h) -> p dc h", dc=4),
                    wkT_d[:, :].rearrange("(dc p) h -> p dc h", p=128),
                )
                nc.sync.dma_start(
                    wvT[:].rearrange("p (dc h) -> p dc h", dc=4),
                    wvT_d[:, :].rearrange("(dc p) h -> p dc h", p=128),
                )
                nc.sync.dma_start(
                    w1vT[:].rearrange("p (dc m) -> p dc m", dc=4),
                    w1vT_d[:, :].rearrange("(dc p) m -> p dc m", p=128),
                )
                nc.sync.dma_start(wr2T[:], wr2T_d[:, :])
                nc.sync.dma_start(bqs[:], bq_d[:, :])
                nc.sync.dma_start(bks[:], bk_d[:, :])
                nc.sync.dma_start(b1c[:], b1c_d[:, :])
                nc.sync.dma_start(bvr[:], bvr_d[:, :])
                nc.sync.dma_start(ones[:], ones_d[:, :])

            def emit_superblock(sb):
                # ---------- A: cast loads ----------
                xbfs = {}
                for name, src, pool in (
                    ("q", xq, ld_pool),
                    ("k", xk, ld_pool),
                    ("v", xv, v_pool),
                ):
                    src_ap = src[bass.ts(sb, ROWS_SB), :].rearrange(
                        "(r p) d -> p r d", p=128
                    )
                    xbf = pool.tile([128, 4 * D], BF16, tag=f"xbf{name}")
                    nc.gpsimd.dma_start(
                        xbf[:].rearrange("p (r d) -> p r d", r=4), src_ap
                    )
                    xbfs[name] = xbf

                # ---------- A2: q/k transposes (one DMA each) ----------
                # xt[p, (r, dc, i)] = x^T[dc*128+p, r*128+i]
                xts = {}
                for name in ("q", "k"):
                    xt = xt_pool.tile([128, 4 * D], BF16, tag=f"xt{name}")
                    nc.sync.dma_start(
                        xt[:].rearrange("p (c i) -> p c i", c=16),
                        xbfs[name][:],
                        transpose=True,
                    )
                    xts[name] = xt

                # ---------- B: q/k projections (bias added on DVE) ----------
                qTs = proj_pool.tile([128, 4 * 512], BF16, tag="qTs")
                kTs = proj_pool.tile([128, 4 * 512], BF16, tag="kTs")
                for dst, w_t, x_t, bias_t in (
                    (qTs, wqT, xts["q"], bqs),
                    (kTs, wkT, xts["k"], bks),
                ):
                    x3 = x_t[:].rearrange("p (r dc i) -> p dc r i", r=4, dc=4)
                    for hc in range(4):
                        ps = psA.tile([128, 512], F32, tag="proj")
                        for dc in range(4):
                            nc.tensor.matmul(
                                ps[:],
                                lhsT=w_t[
                                    :, dc * 512 + hc * 128 : dc * 512 + (hc + 1) * 128
                                ],
                                rhs=x3[:, dc],
                                start=(dc == 0),
                                stop=(dc == 3),
                            )
                        nc.vector.tensor_scalar(
                            dst[:, hc * 512 : (hc + 1) * 512],
                            ps[:],
                            bias_t[:, hc : hc + 1],
                            None,
                            op0=ALU.add,
                        )

                # ---------- C1: scores, staggered partitions ----------
                # pair pr -> partitions (pr%2)*64.., cols pr*64..
                s_all = psS.tile([128, 512], F32, tag="sh")
                for pr in range(SUPER):
                    po = (pr % 2) * 64
                    for hc in range(4):
                        nc.tensor.matmul(
                            s_all[po : po + 64, pr * 64 : (pr + 1) * 64],
                            lhsT=qTs[:, hc * 512 + pr * 64 : hc * 512 + (pr + 1) * 64],
                            rhs=kTs[:, hc * 512 + pr * 64 : hc * 512 + (pr + 1) * 64],
                            start=(hc == 0),
                            stop=(hc == 3),
                        )

                # ---------- C2: E = exp(S) into a zeroed block-diag tile ----
                e_pad = mid_pool.tile([128, 512], BF16, tag="e_pad")
                if sb < 2:
                    nc.vector.memset(e_pad[:], 0.0)
                Zb = mid_pool.tile([128, 8], F32, tag="Zb")
                for pr in range(SUPER):
                    po = (pr % 2) * 64
                    nc.scalar.activation(
                        e_pad[po : po + 64, pr * 64 : (pr + 1) * 64],
                        s_all[po : po + 64, pr * 64 : (pr + 1) * 64],
                        AT.Exp,
                        accum_out=Zb[po : po + 64, pr : pr + 1],
                    )
                rT = mid_pool.tile([128, 8], F32, tag="rT")
                Zb3 = Zb[:].rearrange("p (pp two) -> p pp two", two=2)
                rT3 = rT[:].rearrange("p (pp two) -> p pp two", two=2)
                nc.vector.reciprocal(rT3[0:64, :, 0], Zb3[0:64, :, 0])
                nc.vector.reciprocal(rT3[64:128, :, 1], Zb3[64:128, :, 1])
                for pr in range(SUPER):
                    po = (pr % 2) * 64
                    nc.vector.tensor_scalar(
                        e_pad[po : po + 64, pr * 64 : (pr + 1) * 64],
                        e_pad[po : po + 64, pr * 64 : (pr + 1) * 64],
                        rT[po : po + 64, pr : pr + 1],
                        None,
                        op0=ALU.mult,
                    )

                # ---------- C3: block-diag A^T via one xbar DMA ----------
                # etD[p, pp, q2] = e_pad[q2, pp*128+p]; 2-pair block-diagonal
                etD = mid_pool.tile([128, 4 * 128], BF16, tag="etD")
                nc.sync.dma_start(
                    etD[:].rearrange("p (c i) -> p c i", c=4),
                    e_pad[:],
                    transpose=True,
                )
                et3 = etD[:].rearrange("p (pp q) -> p pp q", pp=4)

                # ---------- C4: B^T, 16 full-K two-pair matmuls ----------
                # bTall[p, (dc, pr, q)] with (pr, q) = (pp, q2)
                bTall = mid_pool.tile([128, 4 * 512], BF16, tag="bTall")
                bT3 = bTall[:].rearrange("p (dc prq) -> p dc prq", dc=4)
                xv_t = xbfs["v"]
                for pp in range(4):
                    ctp = psC.tile([128, 512], F32, tag="ct")
                    for dc in range(4):
                        nc.tensor.matmul(
                            ctp[:, dc * 128 : (dc + 1) * 128],
                            lhsT=xv_t[
                                :, pp * 512 + dc * 128 : pp * 512 + (dc + 1) * 128
                            ],
                            rhs=et3[:, pp, :],
                            start=True,
                            stop=True,
                        )
                    dst = bT3[:, :, pp * 128 : (pp + 1) * 128]
                    src3 = ctp[:].rearrange("p (dc q) -> p dc q", dc=4)
                    if pp % 2 == 0:
                        nc.vector.tensor_copy(dst, src3)
                    else:
                        nc.scalar.activation(dst, src3, AT.Copy)

                # ---------- C5/C6: hidT + lrelu ----------
                hid_all = psS.tile([64, 512], F32, tag="sh")
                for dc in range(4):
                    nc.tensor.matmul(
                        hid_all[:],
                        lhsT=w1vT[:, dc * 64 : (dc + 1) * 64],
                        rhs=bTall[:, dc * 512 : (dc + 1) * 512],
                        start=(dc == 0),
                        stop=(dc == 3),
                    )
                ys_all = mid_pool.tile([64, 512], BF16, tag="ys_all")
                nc.scalar.activation(
                    ys_all[:], hid_all[:], AT.Lrelu, bias=b1c[:], alpha=LEAK
                )

                # ---------- C7/C8: w, wh, stacked whr ----------
                bat = psB.tile([128, 128], F32, tag="batch")
                for pr in range(SUPER):
                    nc.tensor.matmul(
                        bat[0:64, pr : pr + 1],
                        lhsT=ys_all[:, pr * 64 : (pr + 1) * 64],
                        rhs=wr2T[:],
                        start=True,
                        stop=True,
                    )
                wh = mid_pool.tile([64, 8], BF16, tag="wh")
                nc.vector.tensor_scalar(
                    wh[:], bat[0:64, 0:8], float(BR2_VAL[0]), None, op0=ALU.add
                )
                wh3 = wh[:].rearrange("q (pp two) -> q pp two", two=2)
                whr = mid_pool.tile([128, 4], BF16, tag="whr")
                nc.vector.tensor_copy(whr[0:64, :], wh3[:, :, 0])
                nc.vector.tensor_copy(whr[64:128, :], wh3[:, :, 1])

                # ---------- C9: g (2 pairs per matmul, full-K) ----------
                for pp in range(4):
                    nc.tensor.matmul(
                        bat[:, 8 + pp : 9 + pp],
                        lhsT=e_pad[:, pp * 128 : (pp + 1) * 128],
                        rhs=whr[:, pp : pp + 1],
                        start=True,
                        stop=True,
                    )
                # ---------- C10: Sw row ----------
                nc.tensor.matmul(
                    bat[0:1, 88:96], lhsT=ones[:], rhs=wh[:], start=True, stop=True
                )
                swS = mid_pool.tile([1, 8], BF16, tag="swS")
                nc.vector.tensor_copy(swS[:], bat[0:1, 88:96])
                # ---------- C11: block-diag G ----------
                gG = mid_pool.tile([128, 32], BF16, tag="gG")
                g3 = gG[:].rearrange("p (r pr) -> p r pr", r=4)
                nc.vector.memset(gG[:], 0.0)
                for r in range(4):
                    nc.vector.tensor_copy(
                        g3[0:64, r, 2 * r : 2 * r + 1], bat[0:64, 8 + r : 9 + r]
                    )
                    nc.vector.tensor_copy(
                        g3[64:128, r, 2 * r + 1 : 2 * r + 2], bat[64:128, 8 + r : 9 + r]
                    )
                # ---------- C12: z = Xv^T G ----------
                for dc in range(4):
                    for r in range(4):
                        nc.tensor.matmul(
                            bat[:, 16 + dc * 8 : 24 + dc * 8],
                            lhsT=xv_t[
                                :, r * 512 + dc * 128 : r * 512 + (dc + 1) * 128
                            ],
                            rhs=g3[:, r, :],
                            start=(r == 0),
                            stop=(r == 3),
                        )
                zS = mid_pool.tile([128, 32], BF16, tag="zS")
                nc.scalar.activation(zS[:], bat[:, 16:48], AT.Copy)
                # ---------- C13: outT = Wv z + bv Sw ----------
                for hc in range(4):
                    dst = bat[:, 48 + hc * 8 : 56 + hc * 8]
                    for dc in range(4):
                        nc.tensor.matmul(
                            dst,
                            lhsT=wvT[
                                :, dc * 512 + hc * 128 : dc * 512 + (hc + 1) * 128
                            ],
                            rhs=zS[:, dc * 8 : (dc + 1) * 8],
                            start=(dc == 0),
                            stop=False,
                        )
                    nc.tensor.matmul(
                        dst,
                        lhsT=bvr[0:1, hc * 128 : (hc + 1) * 128],
                        rhs=swS[:],
                        start=False,
                        stop=True,
                    )
                outTs = out_pool.tile([128, 32], F32, tag="outTs")
                nc.scalar.activation(outTs[:], bat[:, 48:80], AT.Copy)
                # ---------- C14: transpose + store (DMA on DVE queue) ------
                outN = out_pool.tile([32, 128], F32, tag="outN")
                for b in range(4):
                    nc.vector.transpose(
                        outN[0:32, b * 32 : (b + 1) * 32],
                        outTs[b * 32 : (b + 1) * 32, 0:32],
                    )
                nc.scalar.dma_start(
                    out_d[bass.ts(sb, SUPER), :].rearrange(
                        "pr (hc c) -> hc pr c", hc=4
                    ),
                    outN[:],
                )

            def body(_iv=None):
                load_consts()
                for sb in range(n_sb):
                    emit_superblock(sb)

            if repeat == 1:
                body()
            else:
                with tc.For_i(0, repeat, 1) as _iv:
                    body(_iv)

    nc.compile()
    return nc


def prep_in_maps(query, key, value, Wq, bq, Wk, bk, Wv, bv, Wr1, br1, Wr2, br2):
    s = 1.0 / np.sqrt(np.float32(D))
    wqT = (Wq * s).T.astype(BF16_NP).copy()
    wkT = Wk.T.astype(BF16_NP).copy()
    wvT = Wv.T.astype(BF16_NP).copy()
    w1vT = (Wr1 @ Wv).T.astype(BF16_NP).copy()
    wr2T = Wr2.T.astype(BF16_NP).copy()
    bqv = (bq * s).astype(np.float32).reshape(4, 128).T.copy()
    bkv = bk.astype(np.float32).reshape(4, 128).T.copy()
    b1c = (br1 + Wr1 @ bv).astype(np.float32).reshape(64, 1).copy()
    bvr = bv.astype(BF16_NP).reshape(1, D).copy()
    ones = np.ones((64, 1), dtype=BF16_NP)
    BR2_VAL[0] = float(br2[0])

    in_maps = []
    for c in range(NCORES):
        sl = slice(c * BS_SH, (c + 1) * BS_SH)
        in_maps.append(
            {
                "xq": np.ascontiguousarray(query[sl]).reshape(NPAIR * NSHOT, D),
                "xk": np.ascontiguousarray(key[sl]).reshape(NPAIR * NSHOT, D),
                "xv": np.ascontiguousarray(value[sl]).reshape(NPAIR * NSHOT, D),
                "wqT": wqT,
                "wkT": wkT,
                "wvT": wvT,
                "w1vT": w1vT,
                "wr2T": wr2T,
                "bq": bqv,
                "bk": bkv,
                "b1c": b1c,
                "bvr": bvr,
                "ones": ones,
            }
        )
    return in_maps


_nc_cache = {}


def kernel(**inputs):
    in_maps = prep_in_maps(**{k: np.asarray(v) for k, v in inputs.items()})
    key = ("k", 1, BR2_VAL[0])
    if key not in _nc_cache:
        _nc_cache[key] = build_nc(repeat=1)
    nc = _nc_cache[key]
    res = run_bass_kernel_spmd(nc, in_maps, core_ids=list(range(NCORES)))
    outs = [res.results[c]["out"].reshape(BS_SH, NWAY, D) for c in range(NCORES)]
    return np.concatenate(outs, axis=0).astype(np.float32)
